# revision 2
# baseline (speedup 1.0000x reference)
"""Trainium2 Bass kernel for nn_Base_Filter (depthwise 7x7 conv + weight-norm +
1x1 projection residual + leaky-decay-relu), sharded over K=1024 channels
across 8 NeuronCores (128 channels per core).

Math (folded on host):
  y      = x*(1+w_p) + b_p                       (per-channel affine)
  w_eff  = g * v / ||v||_F                       (weight norm, per channel)
  z      = depthwise_conv7x7_valid(y, w_eff)
  out    = where(z>0, 0.9*z, 0.01*z)

Linearity fold: z = conv(x, w_eff)*(1+w_p) + b_p*sum(w_eff), so with
  w2 = 0.9*(1+w_p)*w_eff,  c2 = 0.9*b_p*sum(w_eff)
we get  out = lrelu(conv(x, w2) + c2, alpha=1/90)  elementwise.

Device algorithm: *phase-split banded-Toeplitz matmuls* (v2; replaces the
per-tap diagonal-matmul scheme). Per channel, put (column-phase s in 0..3,
image row a_i in 0..31) on the 128 contraction partitions and (column-phase
s' in 0..3, output row a_o in 0..24) on the 100 output partitions. The
stationary lhsT[(s,a_i),(s',a_o)] = w2[a_i-a_o, 4h+s-s'] encodes up to 49
taps across h=0..2 column-shift passes, so ONE matmul column computes ~49*100
MACs instead of the diagonal scheme's 128. Per channel: 6 matmuls of N=315
(3 shifts x 2 five-block groups over 10 row-blocks of 25 output rows),
accumulating f32 in PSUM; the bias c2 rides a spare all-ones partition row.
ScalarE evacuates two channels per Lrelu activation (PSUM->SBUF bf16).
All data bf16, all DMA runs >= 600B contiguous; host pre/post-packs layouts.

Per-core cost model: PE ~101us, ACT ~91us, DMA ~47MB => ~131us (bound).
"""

import os
import numpy as np
import ml_dtypes

A = 256
B = 256
R = 32
C = 32
K = 1024
KS = 7
NCORES = 8
PCH = 128        # channels per core

PHI = 4          # column phases
AI = 32          # input rows per block (partition sub-dim)
AOB = 25         # output rows per block
NBLK = 10        # row blocks (10*25 = 250 output rows)
NCOL = 65        # stored columns per (block): n in 0..62 plus shift h<=2
NMM = 63         # matmul columns per block
PO = 100         # output partitions: 4 phases * 25 rows
NH = 3           # column-shift passes
NG = 2           # five-block PSUM groups
BLKG = 5         # blocks per group
NPB = BLKG * NMM # 315 matmul columns per group
AO = A - KS + 1  # 250
BO = B - KS + 1  # 250

CH = int(os.environ.get("KRN_CH", "8"))      # channels per DMA chunk
NCHUNK = PCH // CH

BF16 = ml_dtypes.bfloat16

_COMPILED = {}
LAST_RESULTS = None  # BassKernelResults of the most recent run (for test.py)
LAST_NC = None


def _build_nc():
    import concourse.bacc as bacc
    import concourse.mybir as mybir
    import concourse.tile as tile

    f32 = mybir.dt.float32
    bf16 = mybir.dt.bfloat16
    nc = bacc.Bacc("TRN2", target_bir_lowering=False, debug=False, num_devices=NCORES)

    x_d = nc.declare_dram_parameter("xb", [128, PCH, NBLK, NCOL], bf16, isOutput=False)
    w_d = nc.declare_dram_parameter("wb", [128, PCH, NH, PO], bf16, isOutput=False)
    out_d = nc.declare_dram_parameter(
        "out", [PO, PCH, NG * NPB], bf16, isOutput=True
    )

    with tile.TileContext(nc) as tc:
        from contextlib import ExitStack

        with ExitStack() as ctx:
            xpool = ctx.enter_context(tc.tile_pool(name="x", bufs=3))
            wpool = ctx.enter_context(tc.tile_pool(name="w", bufs=3))
            opool = ctx.enter_context(tc.tile_pool(name="o", bufs=3))
            ppool = ctx.enter_context(tc.tile_pool(name="ps", bufs=2, space="PSUM"))

            for ck in range(NCHUNK):
                c0 = ck * CH
                xt = xpool.tile([128, CH, NBLK, NCOL], bf16, tag="xt")
                nc.sync.dma_start(xt[:], x_d[:, c0 : c0 + CH, :, :])
                wt = wpool.tile([128, CH, NH, PO], bf16, tag="wt")
                nc.sync.dma_start(wt[:], w_d[:, c0 : c0 + CH, :, :])
                ot = opool.tile([PO, CH, NG, NPB], bf16, tag="ot")

                for pr in range(CH // 2):
                    ps = ppool.tile([PO, 2, NG, 512], f32, tag="ps")
                    for cl2 in range(2):
                        cl = pr * 2 + cl2
                        for h in range(NH):
                            for g in range(NG):
                                nc.tensor.matmul(
                                    ps[:, cl2, g, 0:NPB],
                                    wt[:, cl, h, :],
                                    xt[:, cl, BLKG * g : BLKG * g + BLKG, h : h + NMM],
                                    start=(h == 0),
                                    stop=(h == NH - 1),
                                )
                    # out = lrelu(psum), alpha = 0.01/0.9 (0.9 folded in w2/c2)
                    nc.scalar.activation(
                        ot[:, 2 * pr : 2 * pr + 2, :, :],
                        ps[:, :, :, 0:NPB],
                        mybir.ActivationFunctionType.Lrelu,
                        bias=0.0,
                        scale=1.0,
                        alpha=0.01 / 0.9,
                    )

                nc.sync.dma_start(out_d[:, c0 : c0 + CH, :], ot[:])

    nc.compile()
    return nc


def _prep_weights(w_p, b_p, v, g):
    v = v.astype(np.float32)
    v_norm = np.sqrt((v * v).sum(axis=(1, 2), keepdims=True))
    w_eff = g[:, None, None].astype(np.float32) * v / v_norm          # [K,7,7]
    w2 = 0.9 * (1.0 + w_p)[:, None, None].astype(np.float32) * w_eff  # [K,7,7]
    c2 = (0.9 * b_p.astype(np.float32) * w_eff.sum(axis=(1, 2)))      # [K]
    return w2.astype(np.float32), c2.astype(np.float32)


# Index maps for the host-side phase/block gather (module-level constants).
_R_IDX = 25 * np.arange(NBLK)[None, :] + np.arange(AI)[:, None]   # [32, 10]
_C_IDX = 4 * np.arange(NCOL)[None, :] + np.arange(PHI)[:, None]   # [4, 65]


def _pack_x_core(x_t_core):
    """[128, 256, 256] f32 -> partition-major [128p, 128c, 10, 65] bf16."""
    xp = np.zeros((PCH, 257, 260), dtype=np.float32)
    xp[:, :A, :B] = x_t_core
    t1 = xp[:, _R_IDX, :]                    # [128c, 32ai, 10blk, 260]
    t2 = t1[:, :, :, _C_IDX]                 # [128c, 32ai, 10blk, 4s, 65n]
    xbc = np.ascontiguousarray(
        t2.transpose(0, 3, 1, 2, 4)          # [c, s, ai, blk, n]
    ).reshape(PCH, 128, NBLK, NCOL)
    xbc[:, 127, :, :] = 1.0                  # spare row carries the bias ones
    return np.ascontiguousarray(xbc.astype(BF16).transpose(1, 0, 2, 3))


def _pack_w_core(w2c, c2c):
    """[128,7,7], [128] f32 -> partition-major [128p, 128c, 3, 100] bf16."""
    wb = np.zeros((PCH, 128, NH, PO), dtype=np.float32)  # [c, p, h, po]
    ao = np.arange(AOB)
    for di in range(KS):
        for dj in range(KS):
            for sp in range(PHI):
                t = sp + dj
                h, s = divmod(t, PHI)
                wb[:, 32 * s + ao + di, h, 25 * sp + ao] = w2c[:, di, dj][:, None]
    wb[:, 127, 0, :] = c2c[:, None]          # bias via the all-ones rhs row
    return np.ascontiguousarray(wb.astype(BF16).transpose(1, 0, 2, 3))


def _unpack_out_core(buf):
    """[100p, 128c, 630] bf16 -> [128c, 250, 250] f32."""
    b = np.asarray(buf).astype(np.float32).transpose(1, 0, 2)   # [c, po, 630]
    b = b.reshape(PCH, PHI, AOB, NG, BLKG, NMM)                 # [c, s', ao, g, blk, n]
    b = b.transpose(0, 3, 4, 2, 5, 1)                           # [c, g, blk, ao, n, s']
    return np.ascontiguousarray(b.reshape(PCH, AO, NMM * PHI)[:, :, :BO])


def kernel(x, w_p, b_p, v, g):
    global LAST_RESULTS, LAST_NC
    from concourse.bass_utils import run_bass_kernel_spmd

    x = np.asarray(x, dtype=np.float32)
    w2, c2 = _prep_weights(
        np.asarray(w_p, np.float32),
        np.asarray(b_p, np.float32),
        np.asarray(v, np.float32),
        np.asarray(g, np.float32),
    )

    # channel-major x: [K, A, B], k = r*C + c (matches reference's kernel_index)
    x_t = np.ascontiguousarray(x.transpose(2, 3, 0, 1).reshape(K, A, B))

    in_maps = []
    for core in range(NCORES):
        sl = slice(core * PCH, (core + 1) * PCH)
        in_maps.append(
            {
                "xb": _pack_x_core(x_t[sl]),
                "wb": _pack_w_core(w2[sl], c2[sl]),
            }
        )

    key = ("v2", CH)
    if key not in _COMPILED:
        _COMPILED[key] = _build_nc()
    nc = _COMPILED[key]
    LAST_NC = nc

    trace = os.environ.get("KRN_TRACE", "0") == "1"
    res = run_bass_kernel_spmd(nc, in_maps, list(range(NCORES)), trace=trace)
    LAST_RESULTS = res

    out_full = np.empty((K, AO, BO), dtype=np.float32)
    for core in range(NCORES):
        out_full[core * PCH : (core + 1) * PCH] = _unpack_out_core(
            res.results[core]["out"]
        )

    # [K, AO, BO] -> [AO, BO, R, C]
    return np.ascontiguousarray(
        out_full.reshape(R, C, AO, BO).transpose(2, 3, 0, 1)
    )


if __name__ == "__main__":
    rng = np.random.default_rng(0)
    xs = rng.standard_normal((A, B, R, C), dtype=np.float32)
    out = kernel(
        xs,
        rng.standard_normal(K).astype(np.float32) * 0.1,
        rng.standard_normal(K).astype(np.float32) * 0.1,
        rng.standard_normal((K, KS, KS)).astype(np.float32),
        rng.standard_normal(K).astype(np.float32),
    )
    print(out.shape, out.dtype)


# revision 63
# speedup vs baseline: 1.4088x; 1.4088x over previous
"""Trainium2 Bass kernel for nn_Base_Filter (depthwise 7x7 conv + weight-norm +
1x1 projection residual + leaky-decay-relu), sharded over K=1024 channels
across 8 NeuronCores (128 channels per core).

Math (folded on host):
  y      = x*(1+w_p) + b_p                       (per-channel affine)
  w_eff  = g * v / ||v||_F                       (weight norm, per channel)
  z      = depthwise_conv7x7_valid(y, w_eff)
  out    = where(z>0, 0.9*z, 0.01*z)

Linearity fold: z = conv(x, w_eff)*(1+w_p) + b_p*sum(w_eff), so with
  w2 = 0.9*(1+w_p)*w_eff,  c2 = 0.9*b_p*sum(w_eff)
we get  out = lrelu(conv(x, w2) + c2, alpha=1/90)  elementwise.

Device algorithm: *phase-split banded-Toeplitz matmuls* (replaces the
per-tap diagonal-matmul scheme, 5 MACs/cycle/partition-column vs 1). Per
channel, put (column-phase s in 0..3, image row a_i in 0..30) on 124+1
contraction partitions (p = 31*s + a_i; p=124 is an all-ones row carrying
the bias) and (column-phase s', output row a_o in 0..24) on 100 output
partitions. The stationary lhsT[(s,a_i),(s',a_o)] = w2[a_i-a_o, 4h+s-s']
encodes the 49 taps across h=0..2 rhs column-shift passes (the h=2 pass
only has taps for s' in {2,3}, so s' is stored in order [2,3,0,1] and that
lhsT keeps just 50 columns). Per channel: 6 matmuls of N=315 (3 shifts x 2
five-block PSUM groups over 10 row-blocks of 25 output rows) accumulate
f32 in PSUM; ScalarE evacuates two channels per Lrelu activation
(PSUM->SBUF bf16). Everything is bf16, every DMA run is >= 600B
contiguous (host pre/post-packs all layouts, including the 1.25x row-block
duplication). Output stores issue from the Activation engine's HWDGE queue
so input prefetches (SP queue) never block behind ACT-gated stores; 4-
channel priming/tail chunks shorten the serial head/tail.

Per-core model: DMA 44.9MB -> 124.8us (binding), PE 107us, ACT 80us;
TimelineSim 128.3us vs 907.9us baseline (7.1x). HW rel err 3.1e-3.
"""

import os
import numpy as np
import ml_dtypes

A = 256
B = 256
R = 32
C = 32
K = 1024
KS = 7
NCORES = 8
PCH = 128        # channels per core

PHI = 4          # column phases
AI = 32          # input rows per block (partition sub-dim)
AOB = 25         # output rows per block
NBLK = 10        # row blocks (10*25 = 250 output rows)
NCOL = 65        # stored columns per (block): n in 0..62 plus shift h<=2
NMM = 63         # matmul columns per block
PO = 100         # output partitions: 4 phases * 25 rows
NH = 3           # column-shift passes
NG = 2           # five-block PSUM groups
BLKG = 5         # blocks per group
NPB = BLKG * NMM # 315 matmul columns per group
AO = A - KS + 1  # 250
BO = B - KS + 1  # 250

# Output-phase (s') storage order [2, 3, 0, 1]: the h=2 shift pass only has
# taps for s' in {2, 3}, so with those phases first its lhsT needs just the
# first 50 po columns. Weight free-dim layout: h0 -> [0:100], h1 ->
# [100:200], h2 -> [200:250].
_SP_POS = {2: 0, 3: 1, 0: 2, 1: 3}   # s' -> 25-row group position
_SP_INV = [2, 3, 0, 1]               # group position -> s'
WFREE = 250                          # 100 + 100 + 50
_H_OFF = [0, 100, 200]
_H_PO = [100, 100, 50]

NP = 125         # used contraction partitions: 4 phases * 31 rows + ones row
ONES_P = 124     # all-ones partition row (carries the bias via lhsT)

CH = int(os.environ.get("KRN_CH", "8"))      # channels per steady-state chunk
# Small priming/tail chunks shorten the serial head (first compute waits on
# the first x+w DMA) and the serial tail (last out-DMA waits on last ACT).
CHUNKS = [4, 4] + [CH] * ((PCH - 16) // CH) + [4, 4]
assert sum(CHUNKS) == PCH

BF16 = ml_dtypes.bfloat16

_COMPILED = {}
LAST_RESULTS = None  # BassKernelResults of the most recent run (for test.py)
LAST_NC = None


def _build_nc():
    import concourse.bacc as bacc
    import concourse.mybir as mybir
    import concourse.tile as tile

    f32 = mybir.dt.float32
    bf16 = mybir.dt.bfloat16
    nc = bacc.Bacc("TRN2", target_bir_lowering=False, debug=False, num_devices=NCORES)

    x_d = nc.declare_dram_parameter("xb", [NP, PCH, NBLK, NCOL], bf16, isOutput=False)
    w_d = nc.declare_dram_parameter("wb", [NP, PCH, WFREE], bf16, isOutput=False)
    out_d = nc.declare_dram_parameter(
        "out", [PO, PCH, NG * NPB], bf16, isOutput=True
    )

    with tile.TileContext(nc) as tc:
        from contextlib import ExitStack

        with ExitStack() as ctx:
            xpool = ctx.enter_context(tc.tile_pool(name="x", bufs=4))
            wpool = ctx.enter_context(tc.tile_pool(name="w", bufs=4))
            opool = ctx.enter_context(tc.tile_pool(name="o", bufs=4))
            ppool = ctx.enter_context(tc.tile_pool(name="ps", bufs=2, space="PSUM"))

            c0 = 0
            for ch in CHUNKS:
                xt = xpool.tile([NP, ch, NBLK, NCOL], bf16, tag="xt")
                nc.sync.dma_start(xt[:], x_d[:, c0 : c0 + ch, :, :])
                wt = wpool.tile([NP, ch, WFREE], bf16, tag="wt")
                nc.sync.dma_start(wt[:], w_d[:, c0 : c0 + ch, :])
                ot = opool.tile([PO, ch, NG, NPB], bf16, tag="ot")

                for pr in range(ch // 2):
                    ps = ppool.tile([PO, 2, NG, 512], f32, tag="ps")
                    for cl2 in range(2):
                        cl = pr * 2 + cl2
                        for h in range(NH):
                            npo = _H_PO[h]
                            off = _H_OFF[h]
                            for g in range(NG):
                                nc.tensor.matmul(
                                    ps[0:npo, cl2, g, 0:NPB],
                                    wt[:, cl, off : off + npo],
                                    xt[:, cl, BLKG * g : BLKG * g + BLKG, h : h + NMM],
                                    start=(h == 0),
                                    stop=(h == NH - 1),
                                )
                    # out = lrelu(psum), alpha = 0.01/0.9 (0.9 folded in w2/c2)
                    nc.scalar.activation(
                        ot[:, 2 * pr : 2 * pr + 2, :, :],
                        ps[:, :, :, 0:NPB],
                        mybir.ActivationFunctionType.Lrelu,
                        bias=0.0,
                        scale=1.0,
                        alpha=0.01 / 0.9,
                    )

                # Per-chunk ACT-gated output store, issued from the
                # Activation engine's HWDGE queue so input prefetches
                # (SP queue) are never blocked behind it.
                nc.scalar.dma_start(out_d[:, c0 : c0 + ch, :], ot[:])
                c0 += ch

    nc.compile()
    return nc


def _prep_weights(w_p, b_p, v, g):
    v = v.astype(np.float32)
    v_norm = np.sqrt((v * v).sum(axis=(1, 2), keepdims=True))
    w_eff = g[:, None, None].astype(np.float32) * v / v_norm          # [K,7,7]
    w2 = 0.9 * (1.0 + w_p)[:, None, None].astype(np.float32) * w_eff  # [K,7,7]
    c2 = (0.9 * b_p.astype(np.float32) * w_eff.sum(axis=(1, 2)))      # [K]
    return w2.astype(np.float32), c2.astype(np.float32)


# Index maps for the host-side phase/block gather (module-level constants).
_R_IDX = 25 * np.arange(NBLK)[None, :] + np.arange(31)[:, None]   # [31, 10]
_C_IDX = 4 * np.arange(NCOL)[None, :] + np.arange(PHI)[:, None]   # [4, 65]


def _pack_x_core(x_t_core):
    """[128, 256, 256] f32 -> partition-major [125p, 128c, 10, 65] bf16.

    Partition p = 31*s + ai (s = column phase, ai = row-in-block 0..30);
    p = 124 is the all-ones bias row.
    """
    xp = np.zeros((PCH, 257, 260), dtype=np.float32)
    xp[:, :A, :B] = x_t_core
    t1 = xp[:, _R_IDX, :]                    # [128c, 31ai, 10blk, 260]
    t2 = t1[:, :, :, _C_IDX]                 # [128c, 31ai, 10blk, 4s, 65n]
    xbc = np.empty((PCH, NP, NBLK, NCOL), dtype=np.float32)
    xbc[:, :124] = np.ascontiguousarray(
        t2.transpose(0, 3, 1, 2, 4)          # [c, s, ai, blk, n]
    ).reshape(PCH, 124, NBLK, NCOL)
    xbc[:, ONES_P] = 1.0                     # spare row carries the bias ones
    return np.ascontiguousarray(xbc.astype(BF16).transpose(1, 0, 2, 3))


def _pack_w_core(w2c, c2c):
    """[128,7,7], [128] f32 -> partition-major [125p, 128c, 250] bf16."""
    wb = np.zeros((PCH, NP, WFREE), dtype=np.float32)  # [c, p, h-packed po]
    ao = np.arange(AOB)
    for di in range(KS):
        for dj in range(KS):
            for sp in range(PHI):
                t = sp + dj
                h, s = divmod(t, PHI)
                po = _H_OFF[h] + 25 * _SP_POS[sp] + ao
                wb[:, 31 * s + ao + di, po] = w2c[:, di, dj][:, None]
    wb[:, ONES_P, 0:PO] = c2c[:, None]       # bias via the all-ones rhs row
    return np.ascontiguousarray(wb.astype(BF16).transpose(1, 0, 2))


def _unpack_out_core(buf):
    """[100p, 128c, 630] bf16 -> [128c, 250, 250] f32."""
    b = np.asarray(buf).astype(np.float32).transpose(1, 0, 2)   # [c, po, 630]
    b = b.reshape(PCH, PHI, AOB, NG, BLKG, NMM)                 # [c, s'grp, ao, g, blk, n]
    b = b[:, [_SP_POS[0], _SP_POS[1], _SP_POS[2], _SP_POS[3]]]  # ascending s'
    b = b.transpose(0, 3, 4, 2, 5, 1)                           # [c, g, blk, ao, n, s']
    return np.ascontiguousarray(b.reshape(PCH, AO, NMM * PHI)[:, :, :BO])


def kernel(x, w_p, b_p, v, g):
    global LAST_RESULTS, LAST_NC
    from concourse.bass_utils import run_bass_kernel_spmd

    x = np.asarray(x, dtype=np.float32)
    w2, c2 = _prep_weights(
        np.asarray(w_p, np.float32),
        np.asarray(b_p, np.float32),
        np.asarray(v, np.float32),
        np.asarray(g, np.float32),
    )

    # channel-major x: [K, A, B], k = r*C + c (matches reference's kernel_index)
    x_t = np.ascontiguousarray(x.transpose(2, 3, 0, 1).reshape(K, A, B))

    in_maps = []
    for core in range(NCORES):
        sl = slice(core * PCH, (core + 1) * PCH)
        in_maps.append(
            {
                "xb": _pack_x_core(x_t[sl]),
                "wb": _pack_w_core(w2[sl], c2[sl]),
            }
        )

    key = ("v7", CH)
    if key not in _COMPILED:
        _COMPILED[key] = _build_nc()
    nc = _COMPILED[key]
    LAST_NC = nc

    trace = os.environ.get("KRN_TRACE", "0") == "1"
    res = run_bass_kernel_spmd(nc, in_maps, list(range(NCORES)), trace=trace)
    LAST_RESULTS = res

    out_full = np.empty((K, AO, BO), dtype=np.float32)
    for core in range(NCORES):
        out_full[core * PCH : (core + 1) * PCH] = _unpack_out_core(
            res.results[core]["out"]
        )

    # [K, AO, BO] -> [AO, BO, R, C]
    return np.ascontiguousarray(
        out_full.reshape(R, C, AO, BO).transpose(2, 3, 0, 1)
    )


if __name__ == "__main__":
    rng = np.random.default_rng(0)
    xs = rng.standard_normal((A, B, R, C), dtype=np.float32)
    out = kernel(
        xs,
        rng.standard_normal(K).astype(np.float32) * 0.1,
        rng.standard_normal(K).astype(np.float32) * 0.1,
        rng.standard_normal((K, KS, KS)).astype(np.float32),
        rng.standard_normal(K).astype(np.float32),
    )
    print(out.shape, out.dtype)
